# revision 10
# baseline (speedup 1.0000x reference)
"""Trainium2 Bass kernel: single-head attention with QKV projections.

Problem (hardcoded): q/k/v [4,2048,1024] fp32, W_q/W_k/W_v [1024,1024] fp32;
out = softmax((x@Wq^T)(x@Wk^T)^T/32) @ (x@Wv^T), fp32 [4,2048,1024].

Sharding: 8 cores = 4 batches x 2 query-halves, pair-collective K/V
exchange with a permutation-invariant key layout (each core keeps its own
K^T/V half as k-tiles 0..7 and the peer half as k-tiles 8..15; softmax
attention is invariant to key order).

Design (baseline ~191-203us measured; this version targets ~182us):
- S = Q^T K runs in fp8-e4m3 DoubleRow mode (256-deep contraction per
  instruction): W_q/W_k are scaled by 32 on the host so Q,K fill e4m3's
  normal range; exp(S/32768) folds the scales back.  Measured rel-err
  1.76e-2 vs the 2e-2 gate; fp8 anywhere else pushes past the gate
  (CPU-simulated: projections 4.1e-2, attention-V 4.6e-2, mean-centered
  P/V 1.63e-2 alone but 2.1e-2 combined with fp8-S).
- Phase order B'(K proj) -> C'(V proj) -> A(Q proj) -> D(S) -> E(PV).
  K first makes the small fp8 K pair-exchanges the first thing the
  serial CC core (~21us/MB) processes: trace of the V-first ordering
  showed the CC core running V1,V2,K1,K2 (ready-order, not program
  order) with K2 landing 111.8us vs phase D needing it at ~117us --
  a ~6us margin that run-to-run jitter could blow.  K-first gives
  K1/K2 ~50us of margin and V1/V2 still land ~45us before phase E
  reads the peer V tiles.
- The first 3 et-groups of B' are emitted dt-outer over 6 open PSUM
  groups so their matmuls pace the arriving kin/wk input DMAs; the
  dt=0 input tiles are additionally DMA'd in column chunks so the
  first matmul can start ~3us earlier (trace: first DMA trigger fires
  at 8.3us after the framework preamble, first full [128,1024] tile
  lands ~12.3us).
- ~6 warmup matmuls on a memset scratch tile run right after the pool
  barrier: the PE HAM clock gate starts at 1.2GHz and only reaches
  2.4GHz after a ~3.4us busy window, so burning the 8.3->9.5us DMA
  dead-time on dummies moves the warm transition earlier.
- softmax denominators accumulate into one [128,8] PSUM tile (column
  per qt): the old single [128,1] tile serialized qt's first ssum
  matmul behind qt-1's reciprocal read (trace: ~16 ssum matmuls cost a
  full 215ns slot instead of the 27ns shadow slot).
- psum->SBUF copies alternate Vector/Scalar per half (phase E: Vector
  always takes the c=1 half; Scalar's the busier engine).  The last
  qt's epilogue is split into 256-col chunks alternating engines and
  DMA queues to shorten the serial tail after the final matmul.
- Measured floors: projections 3x27.6us, S-phase 26.8us, attention
  ~58.5us, ~8.3us framework preamble, ~3us tail.  Input DMAs are
  full-row (2KB lines; 1KB-line halves measured ~2x slower per byte).
"""

import numpy as np
import ml_dtypes

P = 128
D = 1024
E = 1024
QL = 1024
KL = 2048
KH = 1024
DT, ET, QT, KT = D // P, E // P, QL // P, KL // P
KHT = KH // P

_CACHE = {}


def _build_nc():
    from contextlib import ExitStack

    import concourse.bass as bass
    import concourse.mybir as mybir
    import concourse.tile as tile
    from concourse import bacc

    BF = mybir.dt.bfloat16
    F8 = mybir.dt.float8e4
    F32 = mybir.dt.float32
    AFT = mybir.ActivationFunctionType
    DR = mybir.MatmulPerfMode.DoubleRow

    nc = bacc.Bacc("TRN2", target_bir_lowering=False, debug=False,
                   enable_asserts=False, num_devices=8)

    qinT = nc.dram_tensor("qinT", [D, QL], BF, kind="ExternalInput").ap()
    kinT = nc.dram_tensor("kinT", [D, KH], BF, kind="ExternalInput").ap()
    vinT = nc.dram_tensor("vinT", [D, KH], BF, kind="ExternalInput").ap()
    wqT = nc.dram_tensor("wqT", [D, E], BF, kind="ExternalInput").ap()
    wkT = nc.dram_tensor("wkT", [D, E], BF, kind="ExternalInput").ap()
    wvT = nc.dram_tensor("wvT", [D, E], BF, kind="ExternalInput").ap()
    out = nc.dram_tensor("out", [QL, E], F32, kind="ExternalOutput").ap()

    RG = [[0, 1], [2, 3], [4, 5], [6, 7]]

    with tile.TileContext(nc) as tc, ExitStack() as ctx:
        wpool = ctx.enter_context(tc.tile_pool(name="w", bufs=2))
        apool = ctx.enter_context(tc.tile_pool(name="acts", bufs=2))
        qt_pool = ctx.enter_context(tc.tile_pool(name="qT", bufs=1))
        kt_pool = ctx.enter_context(tc.tile_pool(name="kT", bufs=1))
        v_pool = ctx.enter_context(tc.tile_pool(name="V", bufs=1))
        pt_pool = ctx.enter_context(tc.tile_pool(name="pT", bufs=1))
        o_pool = ctx.enter_context(tc.tile_pool(name="o", bufs=3))
        small = ctx.enter_context(tc.tile_pool(name="small", bufs=1))
        r_pool = ctx.enter_context(tc.tile_pool(name="r", bufs=2))
        ps = ctx.enter_context(tc.tile_pool(name="ps", bufs=7, space="PSUM"))
        ps_s = ctx.enter_context(tc.tile_pool(name="ps_s", bufs=1, space="PSUM"))
        dram = ctx.enter_context(tc.tile_pool(name="dram", bufs=1, space="DRAM"))

        ones_t = small.tile([P, 1], BF, tag="ones")
        nc.vector.memset(ones_t, 1.0)

        # PE warmup: the HAM clock gate needs ~3.4us of busy-ness to lift
        # the PE from 1.2 to 2.4GHz; input DMAs don't land until ~9.5us
        # so burn the dead time on dummy matmuls into a scratch bank.
        # More fillers are woven into B's DMA-paced section below: a ~36%-
        # busy HAM window re-throttles the PE (measured at ts=15.5us), and
        # the cold 427ns matmuls afterwards cost ~2us.
        # The burst must be DENSE: a sparse filler-only stream measured the
        # un-throttle sliding from 12.1us out to 18.0us (SHORT window never
        # majority-busy), turning the whole first projection cold.
        warm_sb = small.tile([P, 512], BF, tag="warm")
        nc.vector.memset(warm_sb, 0.001)
        warm_ps = ps.tile([P, 512], F32, tag="ps", name="warm")
        for i in range(10):
            nc.tensor.matmul(warm_ps, warm_sb[:, 0:P], warm_sb,
                             start=(i == 0), stop=(i == 9))

        qT_sb = qt_pool.tile([P, ET, QL], F8, tag="qT")
        kT_sb = kt_pool.tile([P, ET, KL], F8, tag="kT")
        V_sb = v_pool.tile([P, KT, E], BF, tag="V")
        pT_sb = pt_pool.tile([P, KT, QL], BF, tag="pT")

        cc_in_k1 = dram.tile([4, P, KH], F8, tag="cc_in_k1")
        cc_in_k2 = dram.tile([4, P, KH], F8, tag="cc_in_k2")
        cc_out_k1 = dram.tile([8, P, KH], F8, tag="cc_out_k1")
        cc_out_k2 = dram.tile([8, P, KH], F8, tag="cc_out_k2")
        cc_in_v = dram.tile([KHT, P, E], BF, tag="cc_in_v")
        cc_out_v1 = dram.tile([8, P, E], BF, tag="cc_out_v1")
        cc_out_v2 = dram.tile([8, P, E], BF, tag="cc_out_v2")

        def copy_out(dst, src, use_vector):
            if use_vector:
                nc.vector.tensor_copy(dst, src)
            else:
                nc.scalar.activation(dst, src, AFT.Copy)

        # ---- input DMAs, interleaved across the two HWDGE rings ----
        # wk/kin first: the K^T projection runs first so the small fp8 K
        # pair-exchanges hit the serial CC core before the big V ones.
        # dt=0 tiles are chunked so the first matmul starts ~3us earlier.
        wk_t = [wpool.tile([P, E], BF, tag=f"w{dt}", name=f"wk{dt}")
                for dt in range(DT)]
        kin_t = [apool.tile([P, KH], BF, tag=f"a{dt}", name=f"kin{dt}")
                 for dt in range(DT)]
        nc.sync.dma_start(out=wk_t[0][:, 0:384], in_=wkT[0:P, 0:384])
        nc.scalar.dma_start(out=kin_t[0][:, 0:512], in_=kinT[0:P, 0:512])
        nc.sync.dma_start(out=wk_t[0][:, 384:E], in_=wkT[0:P, 384:E])
        nc.scalar.dma_start(out=kin_t[0][:, 512:KH], in_=kinT[0:P, 512:KH])
        for dt in range(1, DT):
            nc.sync.dma_start(out=wk_t[dt], in_=wkT[dt * P:(dt + 1) * P, :])
            nc.scalar.dma_start(out=kin_t[dt], in_=kinT[dt * P:(dt + 1) * P, :])
        wv_t = [wpool.tile([P, E], BF, tag=f"w{dt}", name=f"wv{dt}")
                for dt in range(DT)]
        vin_t = [apool.tile([P, KH], BF, tag=f"a{dt}", name=f"vin{dt}")
                 for dt in range(DT)]
        for dt in range(DT):
            nc.sync.dma_start(out=wv_t[dt], in_=wvT[dt * P:(dt + 1) * P, :])
            nc.scalar.dma_start(out=vin_t[dt], in_=vinT[dt * P:(dt + 1) * P, :])

        # ---- Phase B': local K^T half -> kT_sb k-tiles 0..7 (fp8) ----
        # The first 3 et groups are emitted dt-outer (6 open PSUM groups)
        # so their matmuls pace the arriving kin/wk tiles instead of the
        # engine FIFO blocking on et0's last contraction step.
        b_acc = {}
        for et in range(3):
            for c in range(2):
                b_acc[(et, c)] = ps.tile([P, 512], F32, tag="ps",
                                         name=f"bacc{et}_{c}")
        for dt in range(DT):
            for et in range(3):
                w_sl = wk_t[dt][:, et * P:(et + 1) * P]
                for c in range(2):
                    nc.tensor.matmul(
                        b_acc[(et, c)], w_sl,
                        kin_t[dt][:, c * 512:(c + 1) * 512],
                        start=(dt == 0), stop=(dt == DT - 1))

        for et in range(3):
            for c in range(2):
                copy_out(kT_sb[:, et, c * 512:(c + 1) * 512], b_acc[(et, c)],
                         (et + c) % 2 == 0)
            nc.scalar.dma_start(out=cc_in_k1[et], in_=kT_sb[:, et, 0:KH])
        for et in range(3, ET):
            acc = [ps.tile([P, 512], F32, tag="ps", name=f"bacc{et}_{c}")
                   for c in range(2)]
            for dt in range(DT):
                w_sl = wk_t[dt][:, et * P:(et + 1) * P]
                for c in range(2):
                    nc.tensor.matmul(
                        acc[c], w_sl, kin_t[dt][:, c * 512:(c + 1) * 512],
                        start=(dt == 0), stop=(dt == DT - 1))
            for c in range(2):
                copy_out(kT_sb[:, et, c * 512:(c + 1) * 512], acc[c],
                         (et + c) % 2 == 0)
            if et < 4:
                nc.scalar.dma_start(out=cc_in_k1[et], in_=kT_sb[:, et, 0:KH])
            else:
                nc.scalar.dma_start(out=cc_in_k2[et - 4],
                                    in_=kT_sb[:, et, 0:KH])
            if et == 3:
                # separate input buffer: a collective reading a tile that
                # later DMAs still write elsewhere into head-of-line blocks
                # the scalar ring (measured +25us)
                nc.gpsimd.collective_compute(
                    "AllGather", mybir.AluOpType.bypass, replica_groups=RG,
                    ins=[cc_in_k1.opt()], outs=[cc_out_k1.opt()])
        nc.gpsimd.collective_compute(
            "AllGather", mybir.AluOpType.bypass, replica_groups=RG,
            ins=[cc_in_k2.opt()], outs=[cc_out_k2.opt()])

        # ---- Phase C': local V half -> V_sb k-tiles 0..7 ----
        # V collective split in two so the CC core starts at kt=3.
        for kt in range(KHT):
            acc = [ps.tile([P, 512], F32, tag="ps", name=f"cacc{kt}_{c}")
                   for c in range(2)]
            for dt in range(DT):
                v_sl = vin_t[dt][:, kt * P:(kt + 1) * P]
                for c in range(2):
                    nc.tensor.matmul(
                        acc[c], v_sl, wv_t[dt][:, c * 512:(c + 1) * 512],
                        start=(dt == 0), stop=(dt == DT - 1))
            for c in range(2):
                copy_out(V_sb[:, kt, c * 512:(c + 1) * 512], acc[c], c == 0)
            nc.scalar.dma_start(out=cc_in_v[kt], in_=V_sb[:, kt, :])
            if kt == 3:
                nc.gpsimd.collective_compute(
                    "AllGather", mybir.AluOpType.bypass, replica_groups=RG,
                    ins=[cc_in_v[0:4].opt()], outs=[cc_out_v1.opt()])
        nc.gpsimd.collective_compute(
            "AllGather", mybir.AluOpType.bypass, replica_groups=RG,
            ins=[cc_in_v[4:8].opt()], outs=[cc_out_v2.opt()])

        # ---- Phase A: Q^T = WqT.T @ qinT (overlaps the collectives) ----
        wq_t = [wpool.tile([P, E], BF, tag=f"w{dt}", name=f"wq{dt}")
                for dt in range(DT)]
        qin_t = [apool.tile([P, QL], BF, tag=f"a{dt}", name=f"qin{dt}")
                 for dt in range(DT)]
        for dt in range(DT):
            nc.sync.dma_start(out=wq_t[dt], in_=wqT[dt * P:(dt + 1) * P, :])
            nc.scalar.dma_start(out=qin_t[dt], in_=qinT[dt * P:(dt + 1) * P, :])
        for et in range(ET):
            acc = [ps.tile([P, 512], F32, tag="ps", name=f"aacc{et}_{c}")
                   for c in range(2)]
            for dt in range(DT):
                w_sl = wq_t[dt][:, et * P:(et + 1) * P]
                for c in range(2):
                    nc.tensor.matmul(
                        acc[c], w_sl, qin_t[dt][:, c * 512:(c + 1) * 512],
                        start=(dt == 0), stop=(dt == DT - 1))
            for c in range(2):
                copy_out(qT_sb[:, et, c * 512:(c + 1) * 512], acc[c], c == 0)

        # ---- unpack the PEER halves into k-tiles 8..15 ----
        # peer block start: 8 if I'm the even rank of the pair, else 0
        pid = nc.sync.partition_id()
        peer_start4 = 4 - (pid % 2) * 4
        src_k1 = cc_out_k1[bass.ds(peer_start4, 4)].rearrange("t p c -> p t c")
        nc.sync.dma_start(out=kT_sb[:, 0:4, KH:KL], in_=src_k1)
        src_k2 = cc_out_k2[bass.ds(peer_start4, 4)].rearrange("t p c -> p t c")
        nc.sync.dma_start(out=kT_sb[:, 4:8, KH:KL], in_=src_k2)
        src_v1 = cc_out_v1[bass.ds(peer_start4, 4)].rearrange("t p c -> p t c")
        nc.sync.dma_start(out=V_sb[:, 8:12, :], in_=src_v1)
        src_v2 = cc_out_v2[bass.ds(peer_start4, 4)].rearrange("t p c -> p t c")
        nc.sync.dma_start(out=V_sb[:, 12:16, :], in_=src_v2)

        # ---- Phase D: S^T (fp8 DoubleRow); P^T = exp(S^T/32768) ----
        # Q' = 32Q, K' = 32K  ->  S'/32768 = QK/32
        for kt in range(KT):
            acc = [ps.tile([P, 512], F32, tag="ps", name=f"dacc{kt}_{c}")
                   for c in range(2)]
            for t in range(ET // 2):
                k_sl = kT_sb[:, 2 * t:2 * t + 2, kt * P:(kt + 1) * P]
                for c in range(2):
                    nc.tensor.matmul(
                        acc[c], k_sl,
                        qT_sb[:, 2 * t:2 * t + 2, c * 512:(c + 1) * 512],
                        start=(t == 0), stop=(t == ET // 2 - 1),
                        perf_mode=DR)
            for c in range(2):
                nc.scalar.activation(pT_sb[:, kt, c * 512:(c + 1) * 512],
                                     acc[c], AFT.Exp, scale=1.0 / 32768.0)

        # ---- Phase E: O' = P^T.T @ V ; s = P^T.T @ 1 ; out = O'/s ----
        for qt in range(QT):
            acc = [ps.tile([P, 512], F32, tag="ps", name=f"eacc{qt}_{c}")
                   for c in range(2)]
            ssum = ps_s.tile([P, 1], F32, tag="ps_s")
            for kt in range(KT):
                p_sl = pT_sb[:, kt, qt * P:(qt + 1) * P]
                # ssum first: the softmax denominator completes two matmuls
                # before the accumulation does, hiding the reciprocal
                nc.tensor.matmul(ssum[:, 0:1], p_sl, ones_t[:, 0:1],
                                 start=(kt == 0), stop=(kt == KT - 1))
                for c in range(2):
                    nc.tensor.matmul(
                        acc[c], p_sl, V_sb[:, kt, c * 512:(c + 1) * 512],
                        start=(kt == 0), stop=(kt == KT - 1))
            r_t = r_pool.tile([P, 1], F32, tag="r")
            nc.vector.reciprocal(r_t, ssum[:, 0:1])
            o_t = o_pool.tile([P, E], F32, tag="o")
            if qt < QT - 1:
                nc.scalar.activation(o_t[:, 0:512], acc[0], AFT.Copy,
                                     scale=r_t[:, 0:1])
                nc.sync.dma_start(out=out[qt * P:(qt + 1) * P, 0:512],
                                  in_=o_t[:, 0:512])
                nc.vector.tensor_scalar_mul(o_t[:, 512:1024], acc[1],
                                            r_t[:, 0:1])
                nc.scalar.dma_start(out=out[qt * P:(qt + 1) * P, 512:1024],
                                    in_=o_t[:, 512:1024])
            else:
                # last qt: 256-col chunks alternating engines and DMA
                # queues to shorten the serial tail after the final matmul
                for h in range(2):
                    cl, ch = h * 256, (h + 1) * 256
                    nc.scalar.activation(o_t[:, cl:ch], acc[0][:, cl:ch],
                                         AFT.Copy, scale=r_t[:, 0:1])
                    nc.sync.dma_start(out=out[qt * P:(qt + 1) * P, cl:ch],
                                      in_=o_t[:, cl:ch])
                    nc.vector.tensor_scalar_mul(o_t[:, 512 + cl:512 + ch],
                                                acc[1][:, cl:ch],
                                                r_t[:, 0:1])
                    nc.scalar.dma_start(
                        out=out[qt * P:(qt + 1) * P, 512 + cl:512 + ch],
                        in_=o_t[:, 512 + cl:512 + ch])

    nc.compile()
    return nc


def _get_nc():
    if "nc" not in _CACHE:
        _CACHE["nc"] = _build_nc()
    return _CACHE["nc"]


def make_in_maps(q, k, v, W_q, W_k, W_v):
    bf = ml_dtypes.bfloat16
    wqT = (np.asarray(W_q, dtype=np.float32) * 32.0).T.astype(bf)
    wkT = (np.asarray(W_k, dtype=np.float32) * 32.0).T.astype(bf)
    wvT = np.asarray(W_v, dtype=np.float32).T.astype(bf)
    in_maps = []
    for c in range(8):
        b, h = c // 2, c % 2
        sl = slice(h * 1024, (h + 1) * 1024)
        in_maps.append({
            "qinT": np.asarray(q[b, sl, :], dtype=np.float32).T.astype(bf),
            "kinT": np.asarray(k[b, sl, :], dtype=np.float32).T.astype(bf),
            "vinT": np.asarray(v[b, sl, :], dtype=np.float32).T.astype(bf),
            "wqT": wqT, "wkT": wkT, "wvT": wvT,
        })
    return in_maps


def kernel(**inputs):
    from concourse import bass_utils

    q = np.asarray(inputs["q_input"], dtype=np.float32)
    k = np.asarray(inputs["k_input"], dtype=np.float32)
    v = np.asarray(inputs["v_input"], dtype=np.float32)

    nc = _get_nc()
    in_maps = make_in_maps(q, k, v, inputs["W_q"], inputs["W_k"], inputs["W_v"])

    res = None
    for attempt in range(3):
        try:
            res = bass_utils.run_bass_kernel_spmd(nc, in_maps,
                                                  core_ids=list(range(8)))
            break
        except Exception:
            if attempt == 2:
                raise
    full = np.empty((4, 2048, 1024), dtype=np.float32)
    for c in range(8):
        b, h = c // 2, c % 2
        full[b, h * 1024:(h + 1) * 1024, :] = res.results[c]["out"]
    return full


# revision 13
# speedup vs baseline: 1.0100x; 1.0100x over previous
"""Trainium2 Bass kernel: single-head attention with QKV projections.

Problem (hardcoded): q/k/v [4,2048,1024] fp32, W_q/W_k/W_v [1024,1024] fp32;
out = softmax((x@Wq^T)(x@Wk^T)^T/32) @ (x@Wv^T), fp32 [4,2048,1024].

Sharding: 8 cores = 4 batches x 2 query-halves, pair-collective K/V
exchange with a permutation-invariant key layout (each core keeps its own
K^T/V half as k-tiles 0..7 and the peer half as k-tiles 8..15; softmax
attention is invariant to key order).

Design (baseline ~191-203us measured; this version targets ~182us):
- S = Q^T K runs in fp8-e4m3 DoubleRow mode (256-deep contraction per
  instruction): W_q/W_k are scaled by 32 on the host so Q,K fill e4m3's
  normal range; exp(S/32768) folds the scales back.  Measured rel-err
  1.76e-2 vs the 2e-2 gate; fp8 anywhere else pushes past the gate
  (CPU-simulated: projections 4.1e-2, attention-V 4.6e-2, mean-centered
  P/V 1.63e-2 alone but 2.1e-2 combined with fp8-S).
- Phase order B'(K proj) -> C'(V proj) -> A(Q proj) -> D(S) -> E(PV).
  K first makes the small fp8 K pair-exchanges the first thing the
  serial CC core (~21us/MB) processes: trace of the V-first ordering
  showed the CC core running V1,V2,K1,K2 (ready-order, not program
  order) with K2 landing 111.8us vs phase D needing it at ~117us --
  a ~6us margin that run-to-run jitter could blow.  K-first gives
  K1/K2 ~50us of margin and V1/V2 still land ~45us before phase E
  reads the peer V tiles.
- The first 3 et-groups of B' are emitted dt-outer over 6 open PSUM
  groups so their matmuls pace the arriving kin/wk input DMAs; the
  dt=0 input tiles are additionally DMA'd in column chunks so the
  first matmul can start ~3us earlier (trace: first DMA trigger fires
  at 8.3us after the framework preamble, first full [128,1024] tile
  lands ~12.3us).
- psum->SBUF copies alternate Vector/Scalar per half (phase E: Vector
  always takes the c=1 half; Scalar's the busier engine).  The last
  qt's epilogue is split into 256-col chunks alternating engines and
  DMA queues to shorten the serial tail after the final matmul.
- Measured floors: projections 3x27.6us, S-phase 26.8us, attention
  ~58.5us, ~8.3us framework preamble, ~3us tail.  Input DMAs are
  full-row (2KB lines; 1KB-line halves measured ~2x slower per byte).
"""

import numpy as np
import ml_dtypes

P = 128
D = 1024
E = 1024
QL = 1024
KL = 2048
KH = 1024
DT, ET, QT, KT = D // P, E // P, QL // P, KL // P
KHT = KH // P

_CACHE = {}


def _build_nc():
    from contextlib import ExitStack

    import concourse.bass as bass
    import concourse.mybir as mybir
    import concourse.tile as tile
    from concourse import bacc

    BF = mybir.dt.bfloat16
    F8 = mybir.dt.float8e4
    F32 = mybir.dt.float32
    AFT = mybir.ActivationFunctionType
    DR = mybir.MatmulPerfMode.DoubleRow

    nc = bacc.Bacc("TRN2", target_bir_lowering=False, debug=False,
                   enable_asserts=False, num_devices=8)

    qinT = nc.dram_tensor("qinT", [D, QL], BF, kind="ExternalInput").ap()
    kinT = nc.dram_tensor("kinT", [D, KH], BF, kind="ExternalInput").ap()
    vinT = nc.dram_tensor("vinT", [D, KH], BF, kind="ExternalInput").ap()
    wqT = nc.dram_tensor("wqT", [D, E], BF, kind="ExternalInput").ap()
    wkT = nc.dram_tensor("wkT", [D, E], BF, kind="ExternalInput").ap()
    wvT = nc.dram_tensor("wvT", [D, E], BF, kind="ExternalInput").ap()
    out = nc.dram_tensor("out", [QL, E], F32, kind="ExternalOutput").ap()

    RG = [[0, 1], [2, 3], [4, 5], [6, 7]]

    with tile.TileContext(nc) as tc, ExitStack() as ctx:
        wpool = ctx.enter_context(tc.tile_pool(name="w", bufs=2))
        apool = ctx.enter_context(tc.tile_pool(name="acts", bufs=2))
        qt_pool = ctx.enter_context(tc.tile_pool(name="qT", bufs=1))
        kt_pool = ctx.enter_context(tc.tile_pool(name="kT", bufs=1))
        v_pool = ctx.enter_context(tc.tile_pool(name="V", bufs=1))
        pt_pool = ctx.enter_context(tc.tile_pool(name="pT", bufs=1))
        o_pool = ctx.enter_context(tc.tile_pool(name="o", bufs=3))
        small = ctx.enter_context(tc.tile_pool(name="small", bufs=1))
        r_pool = ctx.enter_context(tc.tile_pool(name="r", bufs=2))
        ps = ctx.enter_context(tc.tile_pool(name="ps", bufs=7, space="PSUM"))
        ps_s = ctx.enter_context(tc.tile_pool(name="ps_s", bufs=1, space="PSUM"))
        dram = ctx.enter_context(tc.tile_pool(name="dram", bufs=1, space="DRAM"))

        ones_t = small.tile([P, 1], BF, tag="ones")
        nc.vector.memset(ones_t, 1.0)

        # NOTE on PE warmup (HAM clock gate): explicit dummy-matmul bursts
        # and per-dt-group fillers were tried and measured NEUTRAL to
        # NEGATIVE (191.2us with a lucky 6-burst vs 193.6-193.9 for the
        # variants): the PE's own preamble delays the first matmul to
        # 9.3-12.6us regardless, the B' start is input-DMA-paced (cold
        # matmuls run in DMA slack), and extra instructions perturb the
        # Tile scheduler (one run interleaved a DMA-stalled real matmul
        # into the burst, splitting it and postponing warm-up to 23us).
        # The natural dense-cold start warms the clock by ~18us, before
        # the PE-bound et-outer section needs it.
        qT_sb = qt_pool.tile([P, ET, QL], F8, tag="qT")
        kT_sb = kt_pool.tile([P, ET, KL], F8, tag="kT")
        V_sb = v_pool.tile([P, KT, E], BF, tag="V")
        pT_sb = pt_pool.tile([P, KT, QL], BF, tag="pT")

        cc_in_k1 = dram.tile([4, P, KH], F8, tag="cc_in_k1")
        cc_in_k2 = dram.tile([4, P, KH], F8, tag="cc_in_k2")
        cc_out_k1 = dram.tile([8, P, KH], F8, tag="cc_out_k1")
        cc_out_k2 = dram.tile([8, P, KH], F8, tag="cc_out_k2")
        cc_in_v = dram.tile([KHT, P, E], BF, tag="cc_in_v")
        cc_out_v1 = dram.tile([8, P, E], BF, tag="cc_out_v1")
        cc_out_v2 = dram.tile([8, P, E], BF, tag="cc_out_v2")

        def copy_out(dst, src, use_vector):
            if use_vector:
                nc.vector.tensor_copy(dst, src)
            else:
                nc.scalar.activation(dst, src, AFT.Copy)

        # ---- input DMAs, interleaved across the two HWDGE rings ----
        # wk/kin first: the K^T projection runs first so the small fp8 K
        # pair-exchanges hit the serial CC core before the big V ones.
        # dt=0 tiles are chunked so the first matmul starts ~3us earlier.
        wk_t = [wpool.tile([P, E], BF, tag=f"w{dt}", name=f"wk{dt}")
                for dt in range(DT)]
        kin_t = [apool.tile([P, KH], BF, tag=f"a{dt}", name=f"kin{dt}")
                 for dt in range(DT)]
        nc.sync.dma_start(out=wk_t[0][:, 0:384], in_=wkT[0:P, 0:384])
        nc.scalar.dma_start(out=kin_t[0][:, 0:512], in_=kinT[0:P, 0:512])
        nc.sync.dma_start(out=wk_t[0][:, 384:E], in_=wkT[0:P, 384:E])
        nc.scalar.dma_start(out=kin_t[0][:, 512:KH], in_=kinT[0:P, 512:KH])
        for dt in range(1, DT):
            nc.sync.dma_start(out=wk_t[dt], in_=wkT[dt * P:(dt + 1) * P, :])
            nc.scalar.dma_start(out=kin_t[dt], in_=kinT[dt * P:(dt + 1) * P, :])
        wv_t = [wpool.tile([P, E], BF, tag=f"w{dt}", name=f"wv{dt}")
                for dt in range(DT)]
        vin_t = [apool.tile([P, KH], BF, tag=f"a{dt}", name=f"vin{dt}")
                 for dt in range(DT)]
        for dt in range(DT):
            nc.sync.dma_start(out=wv_t[dt], in_=wvT[dt * P:(dt + 1) * P, :])
            nc.scalar.dma_start(out=vin_t[dt], in_=vinT[dt * P:(dt + 1) * P, :])

        # ---- Phase B': local K^T half -> kT_sb k-tiles 0..7 (fp8) ----
        # The first 3 et groups are emitted dt-outer (6 open PSUM groups)
        # so their matmuls pace the arriving kin/wk tiles instead of the
        # engine FIFO blocking on et0's last contraction step.
        b_acc = {}
        for et in range(3):
            for c in range(2):
                b_acc[(et, c)] = ps.tile([P, 512], F32, tag="ps",
                                         name=f"bacc{et}_{c}")
        for dt in range(DT):
            for et in range(3):
                w_sl = wk_t[dt][:, et * P:(et + 1) * P]
                for c in range(2):
                    nc.tensor.matmul(
                        b_acc[(et, c)], w_sl,
                        kin_t[dt][:, c * 512:(c + 1) * 512],
                        start=(dt == 0), stop=(dt == DT - 1))

        for et in range(3):
            for c in range(2):
                copy_out(kT_sb[:, et, c * 512:(c + 1) * 512], b_acc[(et, c)],
                         (et + c) % 2 == 0)
            nc.scalar.dma_start(out=cc_in_k1[et], in_=kT_sb[:, et, 0:KH])
        for et in range(3, ET):
            acc = [ps.tile([P, 512], F32, tag="ps", name=f"bacc{et}_{c}")
                   for c in range(2)]
            for dt in range(DT):
                w_sl = wk_t[dt][:, et * P:(et + 1) * P]
                for c in range(2):
                    nc.tensor.matmul(
                        acc[c], w_sl, kin_t[dt][:, c * 512:(c + 1) * 512],
                        start=(dt == 0), stop=(dt == DT - 1))
            for c in range(2):
                copy_out(kT_sb[:, et, c * 512:(c + 1) * 512], acc[c],
                         (et + c) % 2 == 0)
            if et < 4:
                nc.scalar.dma_start(out=cc_in_k1[et], in_=kT_sb[:, et, 0:KH])
            else:
                nc.scalar.dma_start(out=cc_in_k2[et - 4],
                                    in_=kT_sb[:, et, 0:KH])
            if et == 3:
                # separate input buffer: a collective reading a tile that
                # later DMAs still write elsewhere into head-of-line blocks
                # the scalar ring (measured +25us)
                nc.gpsimd.collective_compute(
                    "AllGather", mybir.AluOpType.bypass, replica_groups=RG,
                    ins=[cc_in_k1.opt()], outs=[cc_out_k1.opt()])
        nc.gpsimd.collective_compute(
            "AllGather", mybir.AluOpType.bypass, replica_groups=RG,
            ins=[cc_in_k2.opt()], outs=[cc_out_k2.opt()])

        # ---- Phase C': local V half -> V_sb k-tiles 0..7 ----
        # V collective split in two so the CC core starts at kt=3.
        for kt in range(KHT):
            acc = [ps.tile([P, 512], F32, tag="ps", name=f"cacc{kt}_{c}")
                   for c in range(2)]
            for dt in range(DT):
                v_sl = vin_t[dt][:, kt * P:(kt + 1) * P]
                for c in range(2):
                    nc.tensor.matmul(
                        acc[c], v_sl, wv_t[dt][:, c * 512:(c + 1) * 512],
                        start=(dt == 0), stop=(dt == DT - 1))
            for c in range(2):
                copy_out(V_sb[:, kt, c * 512:(c + 1) * 512], acc[c], c == 0)
            nc.scalar.dma_start(out=cc_in_v[kt], in_=V_sb[:, kt, :])
            if kt == 3:
                nc.gpsimd.collective_compute(
                    "AllGather", mybir.AluOpType.bypass, replica_groups=RG,
                    ins=[cc_in_v[0:4].opt()], outs=[cc_out_v1.opt()])
        nc.gpsimd.collective_compute(
            "AllGather", mybir.AluOpType.bypass, replica_groups=RG,
            ins=[cc_in_v[4:8].opt()], outs=[cc_out_v2.opt()])

        # ---- Phase A: Q^T = WqT.T @ qinT (overlaps the collectives) ----
        wq_t = [wpool.tile([P, E], BF, tag=f"w{dt}", name=f"wq{dt}")
                for dt in range(DT)]
        qin_t = [apool.tile([P, QL], BF, tag=f"a{dt}", name=f"qin{dt}")
                 for dt in range(DT)]
        for dt in range(DT):
            nc.sync.dma_start(out=wq_t[dt], in_=wqT[dt * P:(dt + 1) * P, :])
            nc.scalar.dma_start(out=qin_t[dt], in_=qinT[dt * P:(dt + 1) * P, :])
        for et in range(ET):
            acc = [ps.tile([P, 512], F32, tag="ps", name=f"aacc{et}_{c}")
                   for c in range(2)]
            for dt in range(DT):
                w_sl = wq_t[dt][:, et * P:(et + 1) * P]
                for c in range(2):
                    nc.tensor.matmul(
                        acc[c], w_sl, qin_t[dt][:, c * 512:(c + 1) * 512],
                        start=(dt == 0), stop=(dt == DT - 1))
            for c in range(2):
                copy_out(qT_sb[:, et, c * 512:(c + 1) * 512], acc[c], c == 0)

        # ---- unpack the PEER halves into k-tiles 8..15 ----
        # peer block start: 8 if I'm the even rank of the pair, else 0
        pid = nc.sync.partition_id()
        peer_start4 = 4 - (pid % 2) * 4
        src_k1 = cc_out_k1[bass.ds(peer_start4, 4)].rearrange("t p c -> p t c")
        nc.sync.dma_start(out=kT_sb[:, 0:4, KH:KL], in_=src_k1)
        src_k2 = cc_out_k2[bass.ds(peer_start4, 4)].rearrange("t p c -> p t c")
        nc.sync.dma_start(out=kT_sb[:, 4:8, KH:KL], in_=src_k2)
        src_v1 = cc_out_v1[bass.ds(peer_start4, 4)].rearrange("t p c -> p t c")
        nc.sync.dma_start(out=V_sb[:, 8:12, :], in_=src_v1)
        src_v2 = cc_out_v2[bass.ds(peer_start4, 4)].rearrange("t p c -> p t c")
        nc.sync.dma_start(out=V_sb[:, 12:16, :], in_=src_v2)

        # ---- Phase D: S^T (fp8 DoubleRow); P^T = exp(S^T/32768) ----
        # Q' = 32Q, K' = 32K  ->  S'/32768 = QK/32
        for kt in range(KT):
            acc = [ps.tile([P, 512], F32, tag="ps", name=f"dacc{kt}_{c}")
                   for c in range(2)]
            for t in range(ET // 2):
                k_sl = kT_sb[:, 2 * t:2 * t + 2, kt * P:(kt + 1) * P]
                for c in range(2):
                    nc.tensor.matmul(
                        acc[c], k_sl,
                        qT_sb[:, 2 * t:2 * t + 2, c * 512:(c + 1) * 512],
                        start=(t == 0), stop=(t == ET // 2 - 1),
                        perf_mode=DR)
            for c in range(2):
                nc.scalar.activation(pT_sb[:, kt, c * 512:(c + 1) * 512],
                                     acc[c], AFT.Exp, scale=1.0 / 32768.0)

        # ---- Phase E: O' = P^T.T @ V ; s = P^T.T @ 1 ; out = O'/s ----
        for qt in range(QT):
            acc = [ps.tile([P, 512], F32, tag="ps", name=f"eacc{qt}_{c}")
                   for c in range(2)]
            ssum = ps_s.tile([P, 1], F32, tag="ps_s")
            for kt in range(KT):
                p_sl = pT_sb[:, kt, qt * P:(qt + 1) * P]
                # ssum first: the softmax denominator completes two matmuls
                # before the accumulation does, hiding the reciprocal
                nc.tensor.matmul(ssum[:, 0:1], p_sl, ones_t[:, 0:1],
                                 start=(kt == 0), stop=(kt == KT - 1))
                for c in range(2):
                    nc.tensor.matmul(
                        acc[c], p_sl, V_sb[:, kt, c * 512:(c + 1) * 512],
                        start=(kt == 0), stop=(kt == KT - 1))
            r_t = r_pool.tile([P, 1], F32, tag="r")
            nc.vector.reciprocal(r_t, ssum[:, 0:1])
            o_t = o_pool.tile([P, E], F32, tag="o")
            if qt < QT - 1:
                nc.scalar.activation(o_t[:, 0:512], acc[0], AFT.Copy,
                                     scale=r_t[:, 0:1])
                nc.sync.dma_start(out=out[qt * P:(qt + 1) * P, 0:512],
                                  in_=o_t[:, 0:512])
                nc.vector.tensor_scalar_mul(o_t[:, 512:1024], acc[1],
                                            r_t[:, 0:1])
                nc.scalar.dma_start(out=out[qt * P:(qt + 1) * P, 512:1024],
                                    in_=o_t[:, 512:1024])
            else:
                # last qt: 256-col chunks alternating engines and DMA
                # queues to shorten the serial tail after the final matmul
                for h in range(2):
                    cl, ch = h * 256, (h + 1) * 256
                    nc.scalar.activation(o_t[:, cl:ch], acc[0][:, cl:ch],
                                         AFT.Copy, scale=r_t[:, 0:1])
                    nc.sync.dma_start(out=out[qt * P:(qt + 1) * P, cl:ch],
                                      in_=o_t[:, cl:ch])
                    nc.vector.tensor_scalar_mul(o_t[:, 512 + cl:512 + ch],
                                                acc[1][:, cl:ch],
                                                r_t[:, 0:1])
                    nc.scalar.dma_start(
                        out=out[qt * P:(qt + 1) * P, 512 + cl:512 + ch],
                        in_=o_t[:, 512 + cl:512 + ch])

    nc.compile()
    return nc


def _get_nc():
    if "nc" not in _CACHE:
        _CACHE["nc"] = _build_nc()
    return _CACHE["nc"]


def make_in_maps(q, k, v, W_q, W_k, W_v):
    bf = ml_dtypes.bfloat16
    wqT = (np.asarray(W_q, dtype=np.float32) * 32.0).T.astype(bf)
    wkT = (np.asarray(W_k, dtype=np.float32) * 32.0).T.astype(bf)
    wvT = np.asarray(W_v, dtype=np.float32).T.astype(bf)
    in_maps = []
    for c in range(8):
        b, h = c // 2, c % 2
        sl = slice(h * 1024, (h + 1) * 1024)
        in_maps.append({
            "qinT": np.asarray(q[b, sl, :], dtype=np.float32).T.astype(bf),
            "kinT": np.asarray(k[b, sl, :], dtype=np.float32).T.astype(bf),
            "vinT": np.asarray(v[b, sl, :], dtype=np.float32).T.astype(bf),
            "wqT": wqT, "wkT": wkT, "wvT": wvT,
        })
    return in_maps


def kernel(**inputs):
    from concourse import bass_utils

    q = np.asarray(inputs["q_input"], dtype=np.float32)
    k = np.asarray(inputs["k_input"], dtype=np.float32)
    v = np.asarray(inputs["v_input"], dtype=np.float32)

    nc = _get_nc()
    in_maps = make_in_maps(q, k, v, inputs["W_q"], inputs["W_k"], inputs["W_v"])

    res = None
    for attempt in range(3):
        try:
            res = bass_utils.run_bass_kernel_spmd(nc, in_maps,
                                                  core_ids=list(range(8)))
            break
        except Exception:
            if attempt == 2:
                raise
    full = np.empty((4, 2048, 1024), dtype=np.float32)
    for c in range(8):
        b, h = c // 2, c % 2
        full[b, h * 1024:(h + 1) * 1024, :] = res.results[c]["out"]
    return full


# revision 14
# speedup vs baseline: 1.0120x; 1.0020x over previous
"""Trainium2 Bass kernel: single-head attention with QKV projections.

Problem (hardcoded): q/k/v [4,2048,1024] fp32, W_q/W_k/W_v [1024,1024] fp32;
out = softmax((x@Wq^T)(x@Wk^T)^T/32) @ (x@Wv^T), fp32 [4,2048,1024].

Sharding: 8 cores = 4 batches x 2 query-halves, pair-collective K/V
exchange with a permutation-invariant key layout (each core keeps its own
K^T/V half as k-tiles 0..7 and the peer half as k-tiles 8..15; softmax
attention is invariant to key order).

Design (baseline ~191-203us measured; this version targets ~182us):
- S = Q^T K runs in fp8-e4m3 DoubleRow mode (256-deep contraction per
  instruction): W_q/W_k are scaled by 32 on the host so Q,K fill e4m3's
  normal range; exp(S/32768) folds the scales back.  Measured rel-err
  1.76e-2 vs the 2e-2 gate; fp8 anywhere else pushes past the gate
  (CPU-simulated: projections 4.1e-2, attention-V 4.6e-2, mean-centered
  P/V 1.63e-2 alone but 2.1e-2 combined with fp8-S).
- Phase order B'(K proj) -> C'(V proj) -> A(Q proj) -> D(S) -> E(PV).
  K first makes the small fp8 K pair-exchanges the first thing the
  serial CC core (~21us/MB) processes: trace of the V-first ordering
  showed the CC core running V1,V2,K1,K2 (ready-order, not program
  order) with K2 landing 111.8us vs phase D needing it at ~117us --
  a ~6us margin that run-to-run jitter could blow.  K-first gives
  K1/K2 ~50us of margin and V1/V2 still land ~45us before phase E
  reads the peer V tiles.
- The first 3 et-groups of B' are emitted dt-outer over 6 open PSUM
  groups so their matmuls pace the arriving kin/wk input DMAs; the
  dt=0 input tiles are additionally DMA'd in column chunks so the
  first matmul can start ~3us earlier (trace: first DMA trigger fires
  at 8.3us after the framework preamble, first full [128,1024] tile
  lands ~12.3us).
- psum->SBUF copies alternate Vector/Scalar per half (phase E: Vector
  always takes the c=1 half; Scalar's the busier engine).  The last
  qt's epilogue is split into 256-col chunks alternating engines and
  DMA queues to shorten the serial tail after the final matmul.
- Measured floors: projections 3x27.6us, S-phase 26.8us, attention
  ~58.5us, ~8.3us framework preamble, ~3us tail.  Input DMAs are
  full-row (2KB lines; 1KB-line halves measured ~2x slower per byte).
"""

import numpy as np
import ml_dtypes

P = 128
D = 1024
E = 1024
QL = 1024
KL = 2048
KH = 1024
DT, ET, QT, KT = D // P, E // P, QL // P, KL // P
KHT = KH // P

_CACHE = {}


def _build_nc():
    from contextlib import ExitStack

    import concourse.bass as bass
    import concourse.mybir as mybir
    import concourse.tile as tile
    from concourse import bacc

    BF = mybir.dt.bfloat16
    F8 = mybir.dt.float8e4
    F32 = mybir.dt.float32
    AFT = mybir.ActivationFunctionType
    DR = mybir.MatmulPerfMode.DoubleRow

    nc = bacc.Bacc("TRN2", target_bir_lowering=False, debug=False,
                   enable_asserts=False, num_devices=8)

    qinT = nc.dram_tensor("qinT", [D, QL], BF, kind="ExternalInput").ap()
    kinT = nc.dram_tensor("kinT", [D, KH], BF, kind="ExternalInput").ap()
    vinT = nc.dram_tensor("vinT", [D, KH], BF, kind="ExternalInput").ap()
    wqT = nc.dram_tensor("wqT", [D, E], BF, kind="ExternalInput").ap()
    wkT = nc.dram_tensor("wkT", [D, E], BF, kind="ExternalInput").ap()
    wvT = nc.dram_tensor("wvT", [D, E], BF, kind="ExternalInput").ap()
    out = nc.dram_tensor("out", [QL, E], F32, kind="ExternalOutput").ap()

    RG = [[0, 1], [2, 3], [4, 5], [6, 7]]

    with tile.TileContext(nc) as tc, ExitStack() as ctx:
        wpool = ctx.enter_context(tc.tile_pool(name="w", bufs=2))
        apool = ctx.enter_context(tc.tile_pool(name="acts", bufs=2))
        qt_pool = ctx.enter_context(tc.tile_pool(name="qT", bufs=1))
        kt_pool = ctx.enter_context(tc.tile_pool(name="kT", bufs=1))
        v_pool = ctx.enter_context(tc.tile_pool(name="V", bufs=1))
        pt_pool = ctx.enter_context(tc.tile_pool(name="pT", bufs=1))
        o_pool = ctx.enter_context(tc.tile_pool(name="o", bufs=3))
        small = ctx.enter_context(tc.tile_pool(name="small", bufs=1))
        r_pool = ctx.enter_context(tc.tile_pool(name="r", bufs=2))
        ps = ctx.enter_context(tc.tile_pool(name="ps", bufs=7, space="PSUM"))
        ps_s = ctx.enter_context(tc.tile_pool(name="ps_s", bufs=1, space="PSUM"))
        dram = ctx.enter_context(tc.tile_pool(name="dram", bufs=1, space="DRAM"))

        ones_t = small.tile([P, 1], BF, tag="ones")
        nc.vector.memset(ones_t, 1.0)

        # NOTE on PE warmup (HAM clock gate): explicit dummy-matmul bursts
        # and per-dt-group fillers were tried and measured NEUTRAL to
        # NEGATIVE (191.2us with a lucky 6-burst vs 193.6-193.9 for the
        # variants): the PE's own preamble delays the first matmul to
        # 9.3-12.6us regardless, the B' start is input-DMA-paced (cold
        # matmuls run in DMA slack), and extra instructions perturb the
        # Tile scheduler (one run interleaved a DMA-stalled real matmul
        # into the burst, splitting it and postponing warm-up to 23us).
        # The natural dense-cold start warms the clock by ~18us, before
        # the PE-bound et-outer section needs it.
        qT_sb = qt_pool.tile([P, ET, QL], F8, tag="qT")
        kT_sb = kt_pool.tile([P, ET, KL], F8, tag="kT")
        V_sb = v_pool.tile([P, KT, E], BF, tag="V")
        pT_sb = pt_pool.tile([P, KT, QL], BF, tag="pT")

        cc_in_k1 = dram.tile([4, P, KH], F8, tag="cc_in_k1")
        cc_in_k2 = dram.tile([4, P, KH], F8, tag="cc_in_k2")
        cc_out_k1 = dram.tile([8, P, KH], F8, tag="cc_out_k1")
        cc_out_k2 = dram.tile([8, P, KH], F8, tag="cc_out_k2")
        cc_in_v = dram.tile([KHT, P, E], BF, tag="cc_in_v")
        cc_out_v1 = dram.tile([8, P, E], BF, tag="cc_out_v1")
        cc_out_v2 = dram.tile([8, P, E], BF, tag="cc_out_v2")

        def copy_out(dst, src, use_vector):
            if use_vector:
                nc.vector.tensor_copy(dst, src)
            else:
                nc.scalar.activation(dst, src, AFT.Copy)

        # ---- input DMAs, interleaved across the two HWDGE rings ----
        # wk/kin first: the K^T projection runs first so the small fp8 K
        # pair-exchanges hit the serial CC core before the big V ones.
        # dt=0 tiles are chunked so the first matmul starts ~3us earlier.
        wk_t = [wpool.tile([P, E], BF, tag=f"w{dt}", name=f"wk{dt}")
                for dt in range(DT)]
        kin_t = [apool.tile([P, KH], BF, tag=f"a{dt}", name=f"kin{dt}")
                 for dt in range(DT)]
        nc.sync.dma_start(out=wk_t[0][:, 0:384], in_=wkT[0:P, 0:384])
        nc.scalar.dma_start(out=kin_t[0][:, 0:512], in_=kinT[0:P, 0:512])
        nc.sync.dma_start(out=wk_t[0][:, 384:E], in_=wkT[0:P, 384:E])
        nc.scalar.dma_start(out=kin_t[0][:, 512:KH], in_=kinT[0:P, 512:KH])
        for dt in range(1, DT):
            nc.sync.dma_start(out=wk_t[dt], in_=wkT[dt * P:(dt + 1) * P, :])
            nc.scalar.dma_start(out=kin_t[dt], in_=kinT[dt * P:(dt + 1) * P, :])
        wv_t = [wpool.tile([P, E], BF, tag=f"w{dt}", name=f"wv{dt}")
                for dt in range(DT)]
        vin_t = [apool.tile([P, KH], BF, tag=f"a{dt}", name=f"vin{dt}")
                 for dt in range(DT)]
        for dt in range(DT):
            nc.sync.dma_start(out=wv_t[dt], in_=wvT[dt * P:(dt + 1) * P, :])
            nc.scalar.dma_start(out=vin_t[dt], in_=vinT[dt * P:(dt + 1) * P, :])

        # ---- Phase B': local K^T half -> kT_sb k-tiles 0..7 (fp8) ----
        # The first 3 et groups are emitted dt-outer (6 open PSUM groups)
        # so their matmuls pace the arriving kin/wk tiles instead of the
        # engine FIFO blocking on et0's last contraction step.
        b_acc = {}
        for et in range(3):
            for c in range(2):
                b_acc[(et, c)] = ps.tile([P, 512], F32, tag="ps",
                                         name=f"bacc{et}_{c}")
        for dt in range(DT):
            for et in range(3):
                w_sl = wk_t[dt][:, et * P:(et + 1) * P]
                for c in range(2):
                    nc.tensor.matmul(
                        b_acc[(et, c)], w_sl,
                        kin_t[dt][:, c * 512:(c + 1) * 512],
                        start=(dt == 0), stop=(dt == DT - 1))

        for et in range(3):
            for c in range(2):
                copy_out(kT_sb[:, et, c * 512:(c + 1) * 512], b_acc[(et, c)],
                         (et + c) % 2 == 0)
            nc.scalar.dma_start(out=cc_in_k1[et], in_=kT_sb[:, et, 0:KH])
        for et in range(3, ET):
            acc = [ps.tile([P, 512], F32, tag="ps", name=f"bacc{et}_{c}")
                   for c in range(2)]
            for dt in range(DT):
                w_sl = wk_t[dt][:, et * P:(et + 1) * P]
                for c in range(2):
                    nc.tensor.matmul(
                        acc[c], w_sl, kin_t[dt][:, c * 512:(c + 1) * 512],
                        start=(dt == 0), stop=(dt == DT - 1))
            for c in range(2):
                copy_out(kT_sb[:, et, c * 512:(c + 1) * 512], acc[c],
                         (et + c) % 2 == 0)
            if et < 4:
                nc.scalar.dma_start(out=cc_in_k1[et], in_=kT_sb[:, et, 0:KH])
            else:
                nc.scalar.dma_start(out=cc_in_k2[et - 4],
                                    in_=kT_sb[:, et, 0:KH])
            if et == 3:
                # separate input buffer: a collective reading a tile that
                # later DMAs still write elsewhere into head-of-line blocks
                # the scalar ring (measured +25us)
                nc.gpsimd.collective_compute(
                    "AllGather", mybir.AluOpType.bypass, replica_groups=RG,
                    ins=[cc_in_k1.opt()], outs=[cc_out_k1.opt()])
        nc.gpsimd.collective_compute(
            "AllGather", mybir.AluOpType.bypass, replica_groups=RG,
            ins=[cc_in_k2.opt()], outs=[cc_out_k2.opt()])

        # ---- Phase C': local V half -> V_sb k-tiles 0..7 ----
        # V collective split in two so the CC core starts at kt=3.
        for kt in range(KHT):
            acc = [ps.tile([P, 512], F32, tag="ps", name=f"cacc{kt}_{c}")
                   for c in range(2)]
            for dt in range(DT):
                v_sl = vin_t[dt][:, kt * P:(kt + 1) * P]
                for c in range(2):
                    nc.tensor.matmul(
                        acc[c], v_sl, wv_t[dt][:, c * 512:(c + 1) * 512],
                        start=(dt == 0), stop=(dt == DT - 1))
            for c in range(2):
                copy_out(V_sb[:, kt, c * 512:(c + 1) * 512], acc[c], c == 0)
            nc.scalar.dma_start(out=cc_in_v[kt], in_=V_sb[:, kt, :])
            if kt == 3:
                nc.gpsimd.collective_compute(
                    "AllGather", mybir.AluOpType.bypass, replica_groups=RG,
                    ins=[cc_in_v[0:4].opt()], outs=[cc_out_v1.opt()])
        nc.gpsimd.collective_compute(
            "AllGather", mybir.AluOpType.bypass, replica_groups=RG,
            ins=[cc_in_v[4:8].opt()], outs=[cc_out_v2.opt()])

        # ---- Phase A: Q^T = WqT.T @ qinT (overlaps the collectives) ----
        wq_t = [wpool.tile([P, E], BF, tag=f"w{dt}", name=f"wq{dt}")
                for dt in range(DT)]
        qin_t = [apool.tile([P, QL], BF, tag=f"a{dt}", name=f"qin{dt}")
                 for dt in range(DT)]
        for dt in range(DT):
            nc.sync.dma_start(out=wq_t[dt], in_=wqT[dt * P:(dt + 1) * P, :])
            nc.scalar.dma_start(out=qin_t[dt], in_=qinT[dt * P:(dt + 1) * P, :])
        for et in range(ET):
            acc = [ps.tile([P, 512], F32, tag="ps", name=f"aacc{et}_{c}")
                   for c in range(2)]
            for dt in range(DT):
                w_sl = wq_t[dt][:, et * P:(et + 1) * P]
                for c in range(2):
                    nc.tensor.matmul(
                        acc[c], w_sl, qin_t[dt][:, c * 512:(c + 1) * 512],
                        start=(dt == 0), stop=(dt == DT - 1))
            for c in range(2):
                copy_out(qT_sb[:, et, c * 512:(c + 1) * 512], acc[c], c == 0)

        # ---- unpack the PEER halves into k-tiles 8..15 ----
        # peer block start: 8 if I'm the even rank of the pair, else 0
        pid = nc.sync.partition_id()
        peer_start4 = 4 - (pid % 2) * 4
        src_k1 = cc_out_k1[bass.ds(peer_start4, 4)].rearrange("t p c -> p t c")
        nc.sync.dma_start(out=kT_sb[:, 0:4, KH:KL], in_=src_k1)
        src_k2 = cc_out_k2[bass.ds(peer_start4, 4)].rearrange("t p c -> p t c")
        nc.sync.dma_start(out=kT_sb[:, 4:8, KH:KL], in_=src_k2)
        src_v1 = cc_out_v1[bass.ds(peer_start4, 4)].rearrange("t p c -> p t c")
        nc.sync.dma_start(out=V_sb[:, 8:12, :], in_=src_v1)
        src_v2 = cc_out_v2[bass.ds(peer_start4, 4)].rearrange("t p c -> p t c")
        nc.sync.dma_start(out=V_sb[:, 12:16, :], in_=src_v2)

        # ---- Phase D: S^T (fp8 DoubleRow); P^T = exp(S^T/32768) ----
        # Q' = 32Q, K' = 32K  ->  S'/32768 = QK/32
        for kt in range(KT):
            acc = [ps.tile([P, 512], F32, tag="ps", name=f"dacc{kt}_{c}")
                   for c in range(2)]
            for t in range(ET // 2):
                k_sl = kT_sb[:, 2 * t:2 * t + 2, kt * P:(kt + 1) * P]
                for c in range(2):
                    nc.tensor.matmul(
                        acc[c], k_sl,
                        qT_sb[:, 2 * t:2 * t + 2, c * 512:(c + 1) * 512],
                        start=(t == 0), stop=(t == ET // 2 - 1),
                        perf_mode=DR)
            for c in range(2):
                nc.scalar.activation(pT_sb[:, kt, c * 512:(c + 1) * 512],
                                     acc[c], AFT.Exp, scale=1.0 / 32768.0)

        # ---- Phase E: O' = P^T.T @ V ; s = P^T.T @ 1 ; out = O'/s ----
        for qt in range(QT):
            acc = [ps.tile([P, 512], F32, tag="ps", name=f"eacc{qt}_{c}")
                   for c in range(2)]
            ssum = ps_s.tile([P, 1], F32, tag="ps_s")
            for kt in range(KT):
                p_sl = pT_sb[:, kt, qt * P:(qt + 1) * P]
                # ssum in the middle of the triple: it shadows in ~27ns
                # behind acc0 (shared stationary), the denominator still
                # completes one matmul before the accumulation does (hiding
                # the reciprocal), and the scheduler is less inclined to
                # defer it out of its kt group (qt0's ssums were measured
                # rescheduled to standalone slots costing ~190ns each when
                # emitted group-first).
                nc.tensor.matmul(acc[0], p_sl, V_sb[:, kt, 0:512],
                                 start=(kt == 0), stop=(kt == KT - 1))
                nc.tensor.matmul(ssum[:, 0:1], p_sl, ones_t[:, 0:1],
                                 start=(kt == 0), stop=(kt == KT - 1))
                nc.tensor.matmul(acc[1], p_sl, V_sb[:, kt, 512:1024],
                                 start=(kt == 0), stop=(kt == KT - 1))
            r_t = r_pool.tile([P, 1], F32, tag="r")
            nc.vector.reciprocal(r_t, ssum[:, 0:1])
            o_t = o_pool.tile([P, E], F32, tag="o")
            if qt < QT - 1:
                nc.scalar.activation(o_t[:, 0:512], acc[0], AFT.Copy,
                                     scale=r_t[:, 0:1])
                nc.sync.dma_start(out=out[qt * P:(qt + 1) * P, 0:512],
                                  in_=o_t[:, 0:512])
                nc.vector.tensor_scalar_mul(o_t[:, 512:1024], acc[1],
                                            r_t[:, 0:1])
                nc.scalar.dma_start(out=out[qt * P:(qt + 1) * P, 512:1024],
                                    in_=o_t[:, 512:1024])
            else:
                # last qt: 256-col chunks alternating engines and DMA
                # queues to shorten the serial tail after the final matmul
                for h in range(2):
                    cl, ch = h * 256, (h + 1) * 256
                    nc.scalar.activation(o_t[:, cl:ch], acc[0][:, cl:ch],
                                         AFT.Copy, scale=r_t[:, 0:1])
                    nc.sync.dma_start(out=out[qt * P:(qt + 1) * P, cl:ch],
                                      in_=o_t[:, cl:ch])
                    nc.vector.tensor_scalar_mul(o_t[:, 512 + cl:512 + ch],
                                                acc[1][:, cl:ch],
                                                r_t[:, 0:1])
                    nc.scalar.dma_start(
                        out=out[qt * P:(qt + 1) * P, 512 + cl:512 + ch],
                        in_=o_t[:, 512 + cl:512 + ch])

    nc.compile()
    return nc


def _get_nc():
    if "nc" not in _CACHE:
        _CACHE["nc"] = _build_nc()
    return _CACHE["nc"]


def make_in_maps(q, k, v, W_q, W_k, W_v):
    bf = ml_dtypes.bfloat16
    wqT = (np.asarray(W_q, dtype=np.float32) * 32.0).T.astype(bf)
    wkT = (np.asarray(W_k, dtype=np.float32) * 32.0).T.astype(bf)
    wvT = np.asarray(W_v, dtype=np.float32).T.astype(bf)
    in_maps = []
    for c in range(8):
        b, h = c // 2, c % 2
        sl = slice(h * 1024, (h + 1) * 1024)
        in_maps.append({
            "qinT": np.asarray(q[b, sl, :], dtype=np.float32).T.astype(bf),
            "kinT": np.asarray(k[b, sl, :], dtype=np.float32).T.astype(bf),
            "vinT": np.asarray(v[b, sl, :], dtype=np.float32).T.astype(bf),
            "wqT": wqT, "wkT": wkT, "wvT": wvT,
        })
    return in_maps


def kernel(**inputs):
    from concourse import bass_utils

    q = np.asarray(inputs["q_input"], dtype=np.float32)
    k = np.asarray(inputs["k_input"], dtype=np.float32)
    v = np.asarray(inputs["v_input"], dtype=np.float32)

    nc = _get_nc()
    in_maps = make_in_maps(q, k, v, inputs["W_q"], inputs["W_k"], inputs["W_v"])

    res = None
    for attempt in range(3):
        try:
            res = bass_utils.run_bass_kernel_spmd(nc, in_maps,
                                                  core_ids=list(range(8)))
            break
        except Exception:
            if attempt == 2:
                raise
    full = np.empty((4, 2048, 1024), dtype=np.float32)
    for c in range(8):
        b, h = c // 2, c % 2
        full[b, h * 1024:(h + 1) * 1024, :] = res.results[c]["out"]
    return full
